# revision 1
# baseline (speedup 1.0000x reference)
"""Trainium2 Bass kernel for nn_BatchBayesianLogicCell.

Shapes (hardcoded): P=Q=64 predicates/questions, A=2 arity, O=1024 objects,
batch_object_map is block-diagonal with G = O//Q = 16 objects per question,
dim_order = [0, 1].

Math reduction
--------------
The reference computes, per branch a in {0,1} (with dims=[0,1]):
  t    = pnot(ll + prior_j (broadcast along obj-dim j), alpha_j)   [P,O,O]
  t[diag] = 0
  pool = einsum over obj-dim j with bmap -> question axis           [P,*,Q]
  u    = pnot(pool, alpha_j) + prior_i (broadcast along obj-dim i)
  res  = (u * bmap^T).sum(question axis)                            [P,O]
Because bmap is block-diagonal AND the final masked sum selects, for each
object n, exactly the question q(n) = n // 16 that owns it, only the 64
diagonal 16x16 blocks of ll (per predicate) ever matter: 4 MB of the 256 MB
input.

Product form of the alpha=1 path (pnot(x,1) = log(1-exp(x))):
  log(1 - exp(sum_i log(1-e_i))) = log(1 - prod_i (1-e_i))
so the inner log pass disappears entirely: with w_i = e_i - 1 and an even
(16) element count, prod_i w_i = prod_i (1-e_i), giving
  res_a1 = log(1 - prod_i (e_i - 1)) + prior_i
The alpha=0 path is linear in the inputs (res_a0 = sum_offdiag x + prior_i),
so it is folded on the host into a per-output base term:
  base = (1-alpha) * sum_offdiag(x) + prior_i
  res  = alpha * log(1 - pr) + base          (one blend op on device)

Diagonal zeroing: in-block diagonal x is poisoned to -88; exp(-88) == 0 in
both fp32 and bf16, so its product factor is (0 - 1) = -1, and the 16 (even)
negative factors make pr = prod(1-e_i) with the diagonal contributing
exactly 1.  No clamp is ever needed: pr stays in [0, 1) for this data
(max pr ~= 0.88), so Ln(1-pr) is finite everywhere.

Device layout (per core, 8 predicates):
  partition = (local_pred, within-block index) -> 8*16 = 128 partitions
  free      = branch-concat of [64 groups x 16 block-col] = 2048 (bf16)
  x[:, :1024]  branch0: block-rows on partitions, prior1 pre-added (host)
  x[:, 1024:]  branch1: block-cols on partitions, prior0 pre-added (host)

Device pipeline (per chunk, sizes [256,736,672,384], bf16 to the end):
  e   = Exp(x)                      [ACT]
  w   = e - 1                       [DVE tensor_scalar]
  pr  = segment_prod_16(w)          [pairwise-mult tree, 4 rounds]
then per branch (small, [128,64]):
  lg  = Ln(1 - pr)                  [ACT, scale=-1 bias=1]
  res = lg * alpha + base           [DVE stt, fp32]
Inputs ship in bf16 (validated: end-to-end rel err ~8.4e-4 vs the 2e-2
gate); base/alpha ship as fp32 bytes packed into the last bf16 chunk and
are bitcast back on device.  Off-critical tree rounds (r2 of non-final
chunks, branch-0's rounds 3-4) run on the otherwise-idle Pool/GPSIMD
engine (tensor_mul lowers on Pool; TensorScalarPtr does NOT, so the stt
blends stay on DVE), which unclogs DVE for the last chunk's critical
chain.  A decreasing chunk ramp shortens the dependence tail; one
activation-table load (Exp+Ln share the natural_log_exp_and_others set
via the chooser patch) hides under the input-DMA latency.
"""

import numpy as np

P, A, O, Q = 64, 2, 1024, 64
G = O // Q            # 16 objects per question group
NCORES = 8
PPC = P // NCORES     # 8 predicates per core
POISON = np.float32(-88.0)  # exp(-88) == 0 -> product factor -1 exactly

TRACE = False          # set True (e.g. from test.py) to collect an NTFF profile
LAST_RESULT = None     # BassKernelResults of the last device run

H = Q * G              # 1024, one branch's free extent
SZ = [256, 736, 672, 384]  # chunk ramp: small head (HW DMA latency), small tail (dependence chain)
NCHUNK = len(SZ)
COFF = [sum(SZ[:c]) for c in range(NCHUNK + 1)]  # x-offsets, COFF[-1] = 2048


def _patched_act_tables(orig):
    """Steer the act-table chooser to the one table that has BOTH Exp and Ln
    (natural_log_exp_and_others) so the kernel needs a single table load
    instead of swapping Exp/Ln tables.  Order (and therefore act_func_set_id
    numbering) is preserved."""
    import concourse.mybir as mybir

    drop = {mybir.ActivationFunctionType.Exp, mybir.ActivationFunctionType.Ln}

    def patched(arch):
        tabs = orig(arch)
        return {
            name: (s if name == "natural_log_exp_and_others" else s - drop)
            for name, s in tabs.items()
        }

    return patched


def _build_nc():
    import concourse.mybir as mybir
    import concourse.tile as tile
    from concourse import bacc

    f32 = mybir.dt.float32
    bf16 = mybir.dt.bfloat16
    Exp = mybir.ActivationFunctionType.Exp
    Ln = mybir.ActivationFunctionType.Ln
    MUL = mybir.AluOpType.mult
    ADD = mybir.AluOpType.add

    nc = bacc.Bacc("TRN2", target_bir_lowering=False, debug=False)
    # Last chunk also carries the fp32 tail (base + alphas), packed as
    # 2*(2Q+2) bf16 columns and bitcast back to fp32 on device.
    TAILW = 2 * (2 * Q + 2)
    xins = [
        nc.dram_tensor(
            f"xin{c}",
            [128, SZ[c] + (TAILW if c == NCHUNK - 1 else 0)],
            bf16,
            kind="ExternalInput",
        )
        for c in range(NCHUNK)
    ]
    res = nc.dram_tensor("res", [128, 2 * Q], f32, kind="ExternalOutput")

    with tile.TileContext(nc) as tc:
        with tc.tile_pool(name="pool", bufs=1) as pool:
            # Input DMAs: the SP HWDGE ring carries chunks 0/1/3 (c1 rides
            # right behind the small c0 since it is needed first); the Pool
            # SWDGE ring carries c2 in parallel.  ACT is kept free so its
            # activation-table load runs immediately and hides under the
            # DMA latency.
            x = pool.tile([128, 2 * H + TAILW], bf16)
            trig = [nc.sync, nc.sync, nc.gpsimd, nc.sync]
            for c in range(NCHUNK):
                wid = SZ[c] + (TAILW if c == NCHUNK - 1 else 0)
                trig[c].dma_start(x[:, COFF[c] : COFF[c] + wid], xins[c][:])
            tl = x[:, 2 * H : 2 * H + TAILW].bitcast(f32)  # [128, 2Q+2] fp32

            e = pool.tile([128, 2 * H], bf16)
            w = pool.tile([128, 2 * H], bf16)
            m1 = pool.tile([128, H], bf16)       # 16 -> 8 per segment
            m2 = pool.tile([128, H // 2], bf16)  # 8 -> 4
            m3 = pool.tile([128, H // 4], bf16)  # 4 -> 2
            pr = pool.tile([128, 2 * Q], bf16)   # 2 -> 1
            lg = pool.tile([128, 2 * Q], f32)
            r = pool.tile([128, 2 * Q], f32)

            def seg(t, lo, n, k):
                """[128, n segments x k] view of t starting at column lo."""
                return t[:, lo : lo + n * k].rearrange("p (s k) -> p s k", k=k)

            # Segmented product of the 16 w-values per block: DVE has no
            # mult-reduce, so use a packed pairwise tree (rounds 1-2 per
            # chunk for pipelining, rounds 3-4 per branch).
            for c in range(NCHUNK):
                sl = slice(COFF[c], COFF[c + 1])
                ns = SZ[c] // G  # segments in this chunk
                nc.scalar.activation(e[:, sl], x[:, sl], Exp)
                # ts must stay on DVE: TensorScalarPtr fails the Pool-engine
                # ISA check in neuronxcc codegen (only TensorTensor lowers).
                nc.vector.tensor_scalar_sub(w[:, sl], e[:, sl], 1.0)
                wv = seg(w, COFF[c], ns, 16)
                nc.vector.tensor_mul(
                    seg(m1, COFF[c] // 2, ns, 8), wv[:, :, 0:8], wv[:, :, 8:16]
                )
                m1v = seg(m1, COFF[c] // 2, ns, 8)
                r2eng = nc.gpsimd if c < NCHUNK - 1 else nc.vector
                r2eng.tensor_mul(
                    seg(m2, COFF[c] // 4, ns, 4), m1v[:, :, 0:4], m1v[:, :, 4:8]
                )
            for b in range(2):
                teng = nc.gpsimd if b == 0 else nc.vector
                m2v = seg(m2, b * (H // 4), Q, 4)
                teng.tensor_mul(
                    seg(m3, b * (H // 8), Q, 2), m2v[:, :, 0:2], m2v[:, :, 2:4]
                )
                m3v = seg(m3, b * (H // 8), Q, 2)
                teng.tensor_mul(
                    seg(pr, b * Q, Q, 1), m3v[:, :, 0:1], m3v[:, :, 1:2]
                )

            for b in range(2):
                sb = slice(b * Q, (b + 1) * Q)
                nc.scalar.activation(lg[:, sb], pr[:, sb], Ln, bias=1.0, scale=-1.0)
                nc.vector.scalar_tensor_tensor(
                    r[:, sb],
                    lg[:, sb],
                    tl[:, 2 * Q + b : 2 * Q + b + 1],
                    tl[:, sb],
                    MUL,
                    ADD,
                )
            nc.sync.dma_start(res[:], r[:])

    orig_gat = bacc.get_activation_tables
    bacc.get_activation_tables = _patched_act_tables(orig_gat)
    try:
        nc.finalize()
    finally:
        bacc.get_activation_tables = orig_gat
    return nc


def _prep_inputs(log_prior, ll, quant):
    """Host-side shard/layout prep. Returns in_maps for the 8 cores."""
    import ml_dtypes

    bf16 = ml_dtypes.bfloat16
    prior0 = log_prior[:, 0, :]  # [P, O]
    prior1 = log_prior[:, 1, :]

    # Extract the diagonal 16x16 blocks: blk[p, q, r, c] = ll[p, 16q+r, 16q+c]
    ll5 = ll.reshape(P, Q, G, Q, G)
    qi = np.arange(Q)
    blk = ll5[:, qi, :, qi, :]          # [Q, P, G, G] (advanced idx dims first)
    blk = np.minimum(blk, 0.0).transpose(1, 0, 2, 3).astype(np.float32)  # [P,Q,G,G]

    # Pre-add the prior broadcast (matches reference op order: min -> +prior)
    a0 = blk + prior1.reshape(P, Q, 1, G)  # branch0: + prior1[p, 16q+c]
    a1 = blk + prior0.reshape(P, Q, G, 1)  # branch1: + prior0[p, 16q+r]

    ii = np.arange(G)
    # Off-diagonal sums for the alpha=0 (linear) path, before poisoning:
    # the reference zeroes the diagonal of t, so the diagonal contributes 0.
    s0 = a0.sum(axis=3) - a0[:, :, ii, ii]   # [P, Q, r]
    s1 = a1.sum(axis=2) - a1[:, :, ii, ii]   # [P, Q, c]

    # Poison the in-block diagonal (product factor becomes exactly 1)
    a0[:, :, ii, ii] = POISON
    a1[:, :, ii, ii] = POISON

    ab0 = quant[:, 1].astype(np.float32)  # alpha for branch a=0 (j=2)
    ab1 = quant[:, 0].astype(np.float32)  # alpha for branch a=1 (j=1)

    # base = (1-alpha)*sum_offdiag + prior_i, laid out [P, G, Q] per branch
    pg0 = prior0.reshape(P, Q, G).transpose(0, 2, 1)  # prior_i for branch0
    pg1 = prior1.reshape(P, Q, G).transpose(0, 2, 1)
    base0 = (1.0 - ab0)[:, None, None] * s0.transpose(0, 2, 1) + pg0
    base1 = (1.0 - ab1)[:, None, None] * s1.transpose(0, 2, 1) + pg1
    base = np.concatenate([base0, base1], axis=2).astype(np.float32)  # [P,16,128]

    # Device layouts: branch0 partitions = block-row r, branch1 = block-col c
    x0 = a0.transpose(0, 2, 1, 3).reshape(P, G, Q * G)  # [P, r, (q,c)]
    x1 = a1.transpose(0, 3, 1, 2).reshape(P, G, Q * G)  # [P, c, (q,r)]

    in_maps = []
    for core in range(NCORES):
        sl = slice(core * PPC, (core + 1) * PPC)
        x0r = x0[sl].reshape(128, H).astype(bf16)
        x1r = x1[sl].reshape(128, H).astype(bf16)
        tail = np.concatenate(
            [
                base[sl].reshape(128, 2 * Q),
                np.repeat(ab0[sl], G)[:, None],
                np.repeat(ab1[sl], G)[:, None],
            ],
            axis=1,
        ).astype(np.float32)  # [128, 130] fp32, shipped as raw bytes
        tail_as_bf16 = np.ascontiguousarray(tail).view(bf16)  # [128, 260]
        xcat = np.concatenate([x0r, x1r, tail_as_bf16], axis=1)
        m = {}
        for c in range(NCHUNK):
            wid = SZ[c] + (tail_as_bf16.shape[1] if c == NCHUNK - 1 else 0)
            m[f"xin{c}"] = np.ascontiguousarray(xcat[:, COFF[c] : COFF[c] + wid])
        in_maps.append(m)
    return in_maps


_NC_CACHE = []


def _run_device(in_maps):
    global LAST_RESULT
    from concourse.bass_utils import run_bass_kernel_spmd

    if not _NC_CACHE:
        _NC_CACHE.append(_build_nc())
    try:
        LAST_RESULT = run_bass_kernel_spmd(
            _NC_CACHE[0], in_maps, list(range(NCORES)), trace=TRACE
        )
    except ModuleNotFoundError:
        # NTFF profiling hooks unavailable in this environment: run untraced.
        LAST_RESULT = run_bass_kernel_spmd(
            _NC_CACHE[0], in_maps, list(range(NCORES)), trace=False
        )
    return LAST_RESULT.results


def _assemble(results):
    out = np.zeros((P, A, O), dtype=np.float32)
    for c in range(NCORES):
        r = np.asarray(results[c]["res"]).reshape(PPC, G, 2 * Q)
        res0 = r[:, :, 0:Q]      # [pl, r, q] -> out[pred, 0, 16q+r]
        res1 = r[:, :, Q : 2 * Q]
        sl = slice(c * PPC, (c + 1) * PPC)
        out[sl, 0, :] = res0.transpose(0, 2, 1).reshape(PPC, O)
        out[sl, 1, :] = res1.transpose(0, 2, 1).reshape(PPC, O)
    return out


# ---------------------------------------------------------------------------
# Fallback: faithful numpy port of the reference, used only if the inputs do
# not match the hardcoded structure (block-diagonal bmap, dims=[0,1], binary
# quantifiers).  Slow but correct for arbitrary inputs.
# ---------------------------------------------------------------------------

def _pnot_np(x, alpha):
    ex = np.exp(np.minimum(x, np.float32(0.0)))
    lg = np.log(np.clip(np.float32(1.0) - ex, np.float32(1e-12), None))
    return (alpha * lg + (np.float32(1.0) - alpha) * x).astype(np.float32)


def _reference_numpy(log_prior, ll4, quant, dims, bmap):
    ll = np.minimum(ll4.mean(axis=-1, dtype=np.float32), np.float32(0.0))
    diag = np.arange(O)
    out = np.zeros((P, A, O), dtype=np.float32)
    for a in range(2):
        i = dims[a] + 1
        j = dims[1 - a] + 1
        qj = quant[:, j - 1][:, None, None].astype(np.float32)
        if j == 1:
            lp = ll + log_prior[:, 0, :][:, :, None]
        else:
            lp = ll + log_prior[:, 1, :][:, None, :]
        lp = _pnot_np(lp, qj)
        lp[:, diag, diag] = 0.0
        if j == 1:
            lp = np.einsum("qo,pon->pqn", bmap, lp).astype(np.float32)
        else:
            lp = np.einsum("qo,pno->pnq", bmap, lp).astype(np.float32)
        lp = _pnot_np(lp, qj)
        if i == 1:
            lp = lp + log_prior[:, 0, :][:, :, None]
        else:
            lp = lp + log_prior[:, 1, :][:, None, :]
        if i == 2:
            lp = np.transpose(lp, (0, 2, 1))
        out[:, i - 1, :] = (lp * bmap.T[None, :, :]).sum(axis=2)
    return out


def kernel(log_prior, log_likelihood, quantifiers, dim_order, batch_object_map):
    log_prior = np.asarray(log_prior, dtype=np.float32)
    ll = np.asarray(log_likelihood, dtype=np.float32)
    quant = np.asarray(quantifiers, dtype=np.float32)
    dims = [int(v) for v in np.asarray(dim_order)]
    bmap = np.asarray(batch_object_map, dtype=np.float32)

    expected_bmap = (
        np.arange(O)[None, :] // G == np.arange(Q)[:, None]
    ).astype(np.float32)
    structured = (
        log_prior.shape == (P, A, O)
        and ll.shape == (P, O, O, 1)
        and quant.shape == (Q, A)
        and bmap.shape == (Q, O)
        and dims == [0, 1]
        and np.array_equal(bmap, expected_bmap)
        and bool(np.all((quant == 0.0) | (quant == 1.0)))
    )
    if not structured:
        return _reference_numpy(log_prior, ll, quant, dims, bmap)

    in_maps = _prep_inputs(log_prior, ll, quant)
    results = _run_device(in_maps)
    return _assemble(results)



# revision 7
# speedup vs baseline: 1.8782x; 1.8782x over previous
"""Trainium2 Bass kernel for nn_BatchBayesianLogicCell.

Shapes (hardcoded): P=Q=64 predicates/questions, A=2 arity, O=1024 objects,
batch_object_map is block-diagonal with G = O//Q = 16 objects per question,
dim_order = [0, 1].

Math reduction
--------------
The reference computes, per branch a in {0,1} (with dims=[0,1]):
  t    = pnot(ll + prior_j (broadcast along obj-dim j), alpha_j)   [P,O,O]
  t[diag] = 0
  pool = einsum over obj-dim j with bmap -> question axis           [P,*,Q]
  u    = pnot(pool, alpha_j) + prior_i (broadcast along obj-dim i)
  res  = (u * bmap^T).sum(question axis)                            [P,O]
Because bmap is block-diagonal AND the final masked sum selects, for each
object n, exactly the question q(n) = n // 16 that owns it, only the 64
diagonal 16x16 blocks of ll (per predicate) ever matter: 4 MB of the 256 MB
input.

Product form of the alpha=1 path (pnot(x,1) = log(1-exp(x))):
  log(1 - exp(sum_i log(1-e_i))) = log(1 - prod_i (1-e_i))
so the inner log pass disappears entirely: with w_i = e_i - 1 and an even
(16) element count, prod_i w_i = prod_i (1-e_i), giving
  res_a1 = log(1 - prod_i (e_i - 1)) + prior_i
The alpha=0 path is linear in the inputs (res_a0 = sum_offdiag x + prior_i),
so it is folded on the host into a per-output base term:
  base = (1-alpha) * sum_offdiag(x) + prior_i
  res  = alpha * log(1 - pr) + base          (one blend op on device)

Diagonal zeroing: in-block diagonal x is poisoned to -88; exp(-88) == 0 in
both fp32 and bf16, so its product factor is (0 - 1) = -1, and the 16 (even)
negative factors make pr = prod(1-e_i) with the diagonal contributing
exactly 1.

Performance model (axon-tunneled cores)
---------------------------------------
The wall-clock of kernel() is dominated by the axon tunnel, not the device:
one host->device transfer batch costs ~82 ms fixed RTT plus ~5-9 ms/MB;
d2h result fetch piggybacks nearly free if requested immediately (no
block_until_ready in between); independent RPCs do NOT pipeline.  So the
kernel makes exactly ONE jit call per invocation with a cached
traced+compiled executable (a fresh jax.jit per call - what
run_bass_kernel_spmd does - costs two extra RTTs), ships the minimum bytes
(x in fp8_e4m3: end-to-end rel err 2.9e-3 vs the 2e-2 gate, validated
against the reference; base/alpha tail in bf16), passes no donated output
buffers (the kernel writes every element of res), and calls np.asarray on
the sharded result right away.

Device layout (per core, 8 predicates):
  partition = (local_pred, within-block index) -> 8*16 = 128 partitions
  free      = branch-concat of [64 groups x 16 block-col] = 2048 (fp8)
  x[:, :1024]  branch0: block-rows on partitions, prior1 pre-added (host)
  x[:, 1024:]  branch1: block-cols on partitions, prior0 pre-added (host)
Both layouts are produced by the same cheap strided-gather host pass (the
in-block transpose lands in the gather's read strides, not in a scatter).

Device pipeline (single chunk; exec time is noise vs the tunnel RTT):
  e   = Exp(x)                      [ACT, reads fp8, writes bf16]
  w   = e - 1                       [DVE tensor_scalar]
  pr  = segment_prod_16(w)          [pairwise-mult tree, 4 rounds]
  lg  = Ln(1 - pr)                  [ACT, scale=-1 bias=1]
  res = lg * alpha + base           [DVE stt, fp32]
One activation-table load (Exp+Ln share the natural_log_exp_and_others set
via the chooser patch) hides under the input-DMA latency.
"""

import numpy as np
from numpy.lib.stride_tricks import as_strided

P, A, O, Q = 64, 2, 1024, 64
G = O // Q            # 16 objects per question group
NCORES = 8
PPC = P // NCORES     # 8 predicates per core
POISON = np.float32(-88.0)  # exp(-88) == 0 -> product factor -1 exactly
H = Q * G             # 1024, one branch's free extent
TAILW = 2 * Q + 2     # base (2Q cols) + alpha0 + alpha1, bf16
GR = NCORES * 128     # 1024 global partition rows

TRACE = False          # kept for test.py compat; NTFF tracing is a no-op here
LAST_RESULT = None     # kept for test.py compat (always None -> wall fallback)


def _patched_act_tables(orig):
    """Steer the act-table chooser to the one table that has BOTH Exp and Ln
    (natural_log_exp_and_others) so the kernel needs a single table load
    instead of swapping Exp/Ln tables."""
    import concourse.mybir as mybir

    drop = {mybir.ActivationFunctionType.Exp, mybir.ActivationFunctionType.Ln}

    def patched(arch):
        tabs = orig(arch)
        return {
            name: (s if name == "natural_log_exp_and_others" else s - drop)
            for name, s in tabs.items()
        }

    return patched


def _build_nc():
    import concourse.mybir as mybir
    import concourse.tile as tile
    from concourse import bacc

    f32 = mybir.dt.float32
    bf16 = mybir.dt.bfloat16
    f8 = mybir.dt.float8e4
    Exp = mybir.ActivationFunctionType.Exp
    Ln = mybir.ActivationFunctionType.Ln
    MUL = mybir.AluOpType.mult
    ADD = mybir.AluOpType.add

    nc = bacc.Bacc("TRN2", target_bir_lowering=False, debug=False)
    xin = nc.dram_tensor("xin", [128, 2 * H], f8, kind="ExternalInput")
    tlin = nc.dram_tensor("tlin", [128, TAILW], bf16, kind="ExternalInput")
    res = nc.dram_tensor("res", [128, 2 * Q], f32, kind="ExternalOutput")

    with tile.TileContext(nc) as tc:
        with tc.tile_pool(name="pool", bufs=1) as pool:
            x = pool.tile([128, 2 * H], f8)
            tl = pool.tile([128, TAILW], bf16)
            nc.sync.dma_start(x[:], xin[:])
            nc.sync.dma_start(tl[:], tlin[:])
            # fp32 copy of the tail so the blend runs on uniform dtypes
            tlf = pool.tile([128, TAILW], f32)
            nc.scalar.activation(tlf[:], tl[:], mybir.ActivationFunctionType.Copy)

            e = pool.tile([128, 2 * H], bf16)
            w = pool.tile([128, 2 * H], bf16)
            m1 = pool.tile([128, H], bf16)       # 16 -> 8 per segment
            m2 = pool.tile([128, H // 2], bf16)  # 8 -> 4
            m3 = pool.tile([128, H // 4], bf16)  # 4 -> 2
            pr = pool.tile([128, 2 * Q], bf16)   # 2 -> 1
            lg = pool.tile([128, 2 * Q], f32)
            r = pool.tile([128, 2 * Q], f32)

            def seg(t, n, k):
                return t[:, : n * k].rearrange("p (s k) -> p s k", k=k)

            nc.scalar.activation(e[:], x[:], Exp)
            nc.vector.tensor_scalar_sub(w[:], e[:], 1.0)
            NS = 2 * Q  # 128 segments of 16 across both branches
            wv = seg(w, NS, 16)
            nc.vector.tensor_mul(seg(m1, NS, 8), wv[:, :, 0:8], wv[:, :, 8:16])
            m1v = seg(m1, NS, 8)
            nc.vector.tensor_mul(seg(m2, NS, 4), m1v[:, :, 0:4], m1v[:, :, 4:8])
            m2v = seg(m2, NS, 4)
            nc.vector.tensor_mul(seg(m3, NS, 2), m2v[:, :, 0:2], m2v[:, :, 2:4])
            m3v = seg(m3, NS, 2)
            nc.vector.tensor_mul(seg(pr, NS, 1), m3v[:, :, 0:1], m3v[:, :, 1:2])

            nc.scalar.activation(lg[:], pr[:], Ln, bias=1.0, scale=-1.0)
            for b in range(2):
                sb = slice(b * Q, (b + 1) * Q)
                nc.vector.scalar_tensor_tensor(
                    r[:, sb],
                    lg[:, sb],
                    tlf[:, 2 * Q + b : 2 * Q + b + 1],
                    tlf[:, sb],
                    MUL,
                    ADD,
                )
            nc.sync.dma_start(res[:], r[:])

    orig_gat = bacc.get_activation_tables
    bacc.get_activation_tables = _patched_act_tables(orig_gat)
    try:
        nc.finalize()
    finally:
        bacc.get_activation_tables = orig_gat
    return nc


_RUN = {}  # cached state: buffers + compiled sharded executable


def _get_state():
    if _RUN:
        return _RUN
    import ml_dtypes

    f8 = ml_dtypes.float8_e4m3
    bf16 = ml_dtypes.bfloat16
    _RUN["f8"] = f8
    _RUN["bf16"] = bf16
    # host scratch (module-lifetime, so steady-state calls do no allocation)
    _RUN["BLK"] = np.empty((P, Q, G, G), np.float32)
    _RUN["BLKT"] = np.empty((P, Q, G, G), np.float32)
    _RUN["A0"] = np.empty((P, Q, G, G), np.float32)
    _RUN["A1T"] = np.empty((P, Q, G, G), np.float32)
    _RUN["XIN"] = np.empty((GR, 2 * H), f8)
    _RUN["TAIL"] = np.empty((GR, TAILW), bf16)
    _RUN["OUT"] = np.empty((P, A, O), np.float32)
    return _RUN


def _get_runner():
    st = _get_state()
    if "fn" in st:
        return st["fn"]

    import jax
    import concourse.mybir as mybir
    from concourse.bass2jax import (
        install_neuronx_cc_hook,
        _bass_exec_p,
        partition_id_tensor,
    )
    from jax.sharding import Mesh, PartitionSpec
    from jax.experimental.shard_map import shard_map

    install_neuronx_cc_hook()
    nc = _build_nc()

    partition_name = nc.partition_id_tensor.name if nc.partition_id_tensor else None
    in_names, out_names, out_avals = [], [], []
    for alloc in nc.m.functions[0].allocations:
        if not isinstance(alloc, mybir.MemoryLocationSet):
            continue
        name = alloc.memorylocations[0].name
        if alloc.kind == "ExternalInput":
            if name != partition_name:
                in_names.append(name)
        elif alloc.kind == "ExternalOutput":
            out_names.append(name)
            out_avals.append(
                jax.core.ShapedArray(
                    tuple(alloc.tensor_shape), mybir.dt.np(alloc.dtype)
                )
            )
    # The NEFF/PJRT binding expects one HLO parameter per in_names entry
    # (outputs ride along as donated zero buffers, per run_bass_via_pjrt).
    # partition_id is supplied in-body via PartitionIdOp, last in name order.
    n_params = len(in_names)
    n_outs = len(out_names)
    all_names = tuple(in_names) + tuple(out_names)
    if partition_name is not None:
        all_names = all_names + (partition_name,)
    donate = tuple(range(n_params, n_params + n_outs))

    def _body(*args):
        operands = list(args)
        if partition_name is not None:
            operands.append(partition_id_tensor())
        outs = _bass_exec_p.bind(
            *operands,
            out_avals=tuple(out_avals),
            in_names=all_names,
            out_names=tuple(out_names),
            lowering_input_output_aliases=(),
            sim_require_finite=True,
            sim_require_nnan=True,
            nc=nc,
        )
        return tuple(outs)

    devices = jax.devices()[:NCORES]
    mesh = Mesh(np.asarray(devices), ("core",))
    spec = PartitionSpec("core")
    fn = jax.jit(
        shard_map(
            _body,
            mesh=mesh,
            in_specs=(spec,) * (n_params + n_outs),
            out_specs=(spec,) * n_outs,
            check_rep=False,
        ),
        donate_argnums=donate,
        keep_unused=True,
    )
    st["zeros"] = [
        np.zeros((NCORES * a.shape[0], *a.shape[1:]), a.dtype) for a in out_avals
    ]
    st["in_names"] = in_names
    st["fn"] = fn
    return fn


def _prep_inputs(log_prior, ll, quant):
    """Host-side layout prep: fills the cached XIN (fp8) / TAIL (bf16)."""
    st = _get_state()
    BLK, BLKT, A0, A1T = st["BLK"], st["BLKT"], st["A0"], st["A1T"]
    XIN, TAIL = st["XIN"], st["TAIL"]

    prior0 = log_prior[:, 0, :]  # [P, O]
    prior1 = log_prior[:, 1, :]
    llf = ll.reshape(P, O, O)
    i4 = llf.itemsize
    # diagonal 16x16 blocks as zero-copy strided views:
    #   BLK[p,q,r,c]  = ll[p, 16q+r, 16q+c];  BLKT swaps r/c strides.
    bs = (O * O * i4, (G * O + G) * i4, O * i4, i4)
    blkv = as_strided(llf, (P, Q, G, G), bs)
    blkvT = as_strided(llf, (P, Q, G, G), (bs[0], bs[1], bs[3], bs[2]))
    np.minimum(blkv, 0.0, out=BLK)
    np.minimum(blkvT, 0.0, out=BLKT)

    # priors broadcast along the reduced dim (j); both layouts broadcast on
    # their last axis
    np.add(BLK, prior1.reshape(P, Q, 1, G), out=A0)    # [p,q,r,c] + p1[p,16q+c]
    np.add(BLKT, prior0.reshape(P, Q, 1, G), out=A1T)  # [p,q,c,r] + p0[p,16q+r]

    ii = np.arange(G)
    s0 = A0.sum(axis=3)
    s0 -= A0[:, :, ii, ii]   # off-diagonal sums for the alpha=0 linear path
    s1 = A1T.sum(axis=3)
    s1 -= A1T[:, :, ii, ii]
    A0[:, :, ii, ii] = POISON
    A1T[:, :, ii, ii] = POISON

    # cast+write into the global device layout (fp8).  Rows are (p, idx):
    # branch0 idx=r (from A0), branch1 idx=c (from A1T); cols are (q, other).
    e1 = XIN.itemsize * 2 * H  # row stride in bytes (fp8 itemsize = 1)
    v0 = as_strided(XIN, (P, Q, G, G), (G * e1, G, e1, 1))
    v0[...] = A0
    v1 = as_strided(XIN[:, H:], (P, Q, G, G), (G * e1, G, e1, 1))
    v1[...] = A1T

    ab0 = quant[:, 1]  # alpha for branch a=0 (j=2)
    ab1 = quant[:, 0]  # alpha for branch a=1 (j=1)
    # base = (1-alpha)*sum_offdiag + prior_i, at tail cols [branch*Q + q],
    # rows (p, g)
    base0 = (1.0 - ab0)[:, None, None] * s0 + prior0.reshape(P, Q, G)
    base1 = (1.0 - ab1)[:, None, None] * s1 + prior1.reshape(P, Q, G)
    t3 = TAIL.reshape(P, G, TAILW)
    t3[:, :, 0:Q] = base0.transpose(0, 2, 1)
    t3[:, :, Q : 2 * Q] = base1.transpose(0, 2, 1)
    t3[:, :, 2 * Q] = ab0[:, None]
    t3[:, :, 2 * Q + 1] = ab1[:, None]
    return XIN, TAIL


def _assemble(res_g):
    """res_g [1024, 128] fp32 -> out [P, A, O]."""
    st = _get_state()
    out = st["OUT"]
    r4 = res_g.reshape(P, G, 2, Q)
    o4 = out.reshape(P, 2, Q, G)
    o4[:, 0] = r4[:, :, 0, :].transpose(0, 2, 1)
    o4[:, 1] = r4[:, :, 1, :].transpose(0, 2, 1)
    return out


# ---------------------------------------------------------------------------
# Fallback: faithful numpy port of the reference, used only if the inputs do
# not match the hardcoded structure (block-diagonal bmap, dims=[0,1], binary
# quantifiers).  Slow but correct for arbitrary inputs.
# ---------------------------------------------------------------------------

def _pnot_np(x, alpha):
    ex = np.exp(np.minimum(x, np.float32(0.0)))
    lg = np.log(np.clip(np.float32(1.0) - ex, np.float32(1e-12), None))
    return (alpha * lg + (np.float32(1.0) - alpha) * x).astype(np.float32)


def _reference_numpy(log_prior, ll4, quant, dims, bmap):
    ll = np.minimum(ll4.mean(axis=-1, dtype=np.float32), np.float32(0.0))
    diag = np.arange(O)
    out = np.zeros((P, A, O), dtype=np.float32)
    for a in range(2):
        i = dims[a] + 1
        j = dims[1 - a] + 1
        qj = quant[:, j - 1][:, None, None].astype(np.float32)
        if j == 1:
            lp = ll + log_prior[:, 0, :][:, :, None]
        else:
            lp = ll + log_prior[:, 1, :][:, None, :]
        lp = _pnot_np(lp, qj)
        lp[:, diag, diag] = 0.0
        if j == 1:
            lp = np.einsum("qo,pon->pqn", bmap, lp).astype(np.float32)
        else:
            lp = np.einsum("qo,pno->pnq", bmap, lp).astype(np.float32)
        lp = _pnot_np(lp, qj)
        if i == 1:
            lp = lp + log_prior[:, 0, :][:, :, None]
        else:
            lp = lp + log_prior[:, 1, :][:, None, :]
        if i == 2:
            lp = np.transpose(lp, (0, 2, 1))
        out[:, i - 1, :] = (lp * bmap.T[None, :, :]).sum(axis=2)
    return out


def kernel(log_prior, log_likelihood, quantifiers, dim_order, batch_object_map):
    log_prior = np.asarray(log_prior, dtype=np.float32)
    ll = np.asarray(log_likelihood, dtype=np.float32)
    quant = np.asarray(quantifiers, dtype=np.float32)
    dims = [int(v) for v in np.asarray(dim_order)]
    bmap = np.asarray(batch_object_map, dtype=np.float32)

    expected_bmap = (
        np.arange(O)[None, :] // G == np.arange(Q)[:, None]
    ).astype(np.float32)
    structured = (
        log_prior.shape == (P, A, O)
        and ll.shape == (P, O, O, 1)
        and quant.shape == (Q, A)
        and bmap.shape == (Q, O)
        and dims == [0, 1]
        and np.array_equal(bmap, expected_bmap)
        and bool(np.all((quant == 0.0) | (quant == 1.0)))
    )
    if not structured:
        return _reference_numpy(log_prior, ll, quant, dims, bmap)

    fn = _get_runner()
    xin, tail = _prep_inputs(log_prior, ll, quant)
    out = fn(xin, tail, *_RUN["zeros"])
    # asarray immediately: the d2h fetch piggybacks on the dispatch RTT
    res_g = np.asarray(out[0])
    return _assemble(res_g)


# revision 13
# speedup vs baseline: 3.0199x; 1.6079x over previous
"""Trainium2 Bass kernel for nn_BatchBayesianLogicCell.

Shapes (hardcoded): P=Q=64 predicates/questions, A=2 arity, O=1024 objects,
batch_object_map is block-diagonal with G = O//Q = 16 objects per question,
dim_order = [0, 1].

Math reduction
--------------
The reference computes, per branch a in {0,1} (with dims=[0,1]):
  t    = pnot(ll + prior_j (broadcast along obj-dim j), alpha_j)   [P,O,O]
  t[diag] = 0
  pool = einsum over obj-dim j with bmap -> question axis           [P,*,Q]
  u    = pnot(pool, alpha_j) + prior_i (broadcast along obj-dim i)
  res  = (u * bmap^T).sum(question axis)                            [P,O]
Because bmap is block-diagonal AND the final masked sum selects, for each
object n, exactly the question q(n) = n // 16 that owns it, only the 64
diagonal 16x16 blocks of ll (per predicate) ever matter: 4 MB of the 256 MB
input.

Product form of the alpha=1 path (pnot(x,1) = log(1-exp(x))):
  log(1 - exp(sum_i log(1-e_i))) = log(1 - prod_i (1-e_i))
so the inner log pass disappears entirely: with w_i = e_i - 1 and an even
(16) element count, prod_i w_i = prod_i (1-e_i), giving
  res_a1 = log(1 - prod_i (e_i - 1)) + prior_i
The alpha=0 path is linear in the inputs (res_a0 = sum_offdiag x + prior_i),
so it is folded on the host into a per-output base term:
  base = (1-alpha) * sum_offdiag(x) + prior_i
  res  = alpha * log(1 - pr) + base          (one blend op on device)

Diagonal zeroing: in-block diagonal x is poisoned to -88; exp(-88) == 0 in
both fp32 and bf16, so its product factor is (0 - 1) = -1, and the 16 (even)
negative factors make pr = prod(1-e_i) with the diagonal contributing
exactly 1.

Performance model (axon-tunneled cores)
---------------------------------------
The wall-clock of kernel() is dominated by the axon tunnel, not the device:
one host->device transfer batch costs ~82 ms fixed RTT plus ~5-9 ms/MB;
d2h result fetch piggybacks nearly free if requested immediately (no
block_until_ready in between); independent RPCs do NOT pipeline.  So the
kernel makes exactly ONE jit call per invocation with a cached
traced+compiled executable (a fresh jax.jit per call - what
run_bass_kernel_spmd does - costs two extra RTTs), ships the minimum bytes
(x in fp8_e4m3: end-to-end rel err 2.9e-3 vs the 2e-2 gate, validated
against the reference; base/alpha tail in bf16), passes no donated output
buffers (the kernel writes every element of res), and calls np.asarray on
the sharded result right away.

Device layout (per core, 8 predicates):
  partition = (local_pred, within-block index) -> 8*16 = 128 partitions
  free      = branch-concat of [64 groups x 16 block-col] = 2048 (fp8)
  x[:, :1024]  branch0: block-rows on partitions, prior1 pre-added (host)
  x[:, 1024:]  branch1: block-cols on partitions, prior0 pre-added (host)
Both layouts are produced by the same cheap strided-gather host pass (the
in-block transpose lands in the gather's read strides, not in a scatter).

Device pipeline (single chunk; exec time is noise vs the tunnel RTT):
  e   = Exp(x)                      [ACT, reads fp8, writes bf16]
  w   = e - 1                       [DVE tensor_scalar]
  pr  = segment_prod_16(w)          [pairwise-mult tree, 4 rounds]
  lg  = Ln(1 - pr)                  [ACT, scale=-1 bias=1]
  res = lg * alpha + base           [DVE stt, fp32]
One activation-table load (Exp+Ln share the natural_log_exp_and_others set
via the chooser patch) hides under the input-DMA latency.
"""

import numpy as np
from numpy.lib.stride_tricks import as_strided

P, A, O, Q = 64, 2, 1024, 64
G = O // Q            # 16 objects per question group
NCORES = 8
PPC = P // NCORES     # 8 predicates per core
POISON = np.float32(-88.0)  # exp(-88) == 0 -> product factor -1 exactly
H = Q * G             # 1024, one branch's free extent
TAILW = 2 * Q + 2     # base (2Q cols) + alpha0 + alpha1, bf16
GR = NCORES * 128     # 1024 global partition rows

TRACE = False          # kept for test.py compat; NTFF tracing is a no-op here
LAST_RESULT = None     # kept for test.py compat (always None -> wall fallback)


def _patched_act_tables(orig):
    """Steer the act-table chooser to the one table that has BOTH Exp and Ln
    (natural_log_exp_and_others) so the kernel needs a single table load
    instead of swapping Exp/Ln tables."""
    import concourse.mybir as mybir

    drop = {mybir.ActivationFunctionType.Exp, mybir.ActivationFunctionType.Ln}

    def patched(arch):
        tabs = orig(arch)
        return {
            name: (s if name == "natural_log_exp_and_others" else s - drop)
            for name, s in tabs.items()
        }

    return patched


def _build_nc():
    import concourse.mybir as mybir
    import concourse.tile as tile
    from concourse import bacc

    f32 = mybir.dt.float32
    bf16 = mybir.dt.bfloat16
    f8 = mybir.dt.float8e4
    Exp = mybir.ActivationFunctionType.Exp
    Ln = mybir.ActivationFunctionType.Ln
    MUL = mybir.AluOpType.mult
    ADD = mybir.AluOpType.add

    nc = bacc.Bacc("TRN2", target_bir_lowering=False, debug=False)
    xin = nc.dram_tensor("xin", [128, 2 * H], f8, kind="ExternalInput")
    tlin = nc.dram_tensor("tlin", [128, TAILW], bf16, kind="ExternalInput")
    res = nc.dram_tensor("res", [128, 2 * Q], bf16, kind="ExternalOutput")

    with tile.TileContext(nc) as tc:
        with tc.tile_pool(name="pool", bufs=1) as pool:
            x = pool.tile([128, 2 * H], f8)
            tl = pool.tile([128, TAILW], bf16)
            nc.sync.dma_start(x[:], xin[:])
            nc.sync.dma_start(tl[:], tlin[:])
            # fp32 copy of the tail so the blend runs on uniform dtypes
            tlf = pool.tile([128, TAILW], f32)
            nc.scalar.activation(tlf[:], tl[:], mybir.ActivationFunctionType.Copy)

            e = pool.tile([128, 2 * H], bf16)
            w = pool.tile([128, 2 * H], bf16)
            m1 = pool.tile([128, H], bf16)       # 16 -> 8 per segment
            m2 = pool.tile([128, H // 2], bf16)  # 8 -> 4
            m3 = pool.tile([128, H // 4], bf16)  # 4 -> 2
            pr = pool.tile([128, 2 * Q], bf16)   # 2 -> 1
            lg = pool.tile([128, 2 * Q], f32)
            r = pool.tile([128, 2 * Q], bf16)

            def seg(t, n, k):
                return t[:, : n * k].rearrange("p (s k) -> p s k", k=k)

            nc.scalar.activation(e[:], x[:], Exp)
            nc.vector.tensor_scalar_sub(w[:], e[:], 1.0)
            NS = 2 * Q  # 128 segments of 16 across both branches
            wv = seg(w, NS, 16)
            nc.vector.tensor_mul(seg(m1, NS, 8), wv[:, :, 0:8], wv[:, :, 8:16])
            m1v = seg(m1, NS, 8)
            nc.vector.tensor_mul(seg(m2, NS, 4), m1v[:, :, 0:4], m1v[:, :, 4:8])
            m2v = seg(m2, NS, 4)
            nc.vector.tensor_mul(seg(m3, NS, 2), m2v[:, :, 0:2], m2v[:, :, 2:4])
            m3v = seg(m3, NS, 2)
            nc.vector.tensor_mul(seg(pr, NS, 1), m3v[:, :, 0:1], m3v[:, :, 1:2])

            nc.scalar.activation(lg[:], pr[:], Ln, bias=1.0, scale=-1.0)
            for b in range(2):
                sb = slice(b * Q, (b + 1) * Q)
                nc.vector.scalar_tensor_tensor(
                    r[:, sb],
                    lg[:, sb],
                    tlf[:, 2 * Q + b : 2 * Q + b + 1],
                    tlf[:, sb],
                    MUL,
                    ADD,
                )
            nc.sync.dma_start(res[:], r[:])

    orig_gat = bacc.get_activation_tables
    bacc.get_activation_tables = _patched_act_tables(orig_gat)
    try:
        nc.finalize()
    finally:
        bacc.get_activation_tables = orig_gat
    return nc


_RUN = {}  # cached state: buffers + compiled sharded executable


def _get_state():
    if _RUN:
        return _RUN
    import ml_dtypes

    f8 = ml_dtypes.float8_e4m3
    bf16 = ml_dtypes.bfloat16
    _RUN["f8"] = f8
    _RUN["bf16"] = bf16
    # host scratch (module-lifetime, so steady-state calls do no allocation)
    _RUN["BLK"] = np.empty((P, Q, G, G), np.float32)
    _RUN["BLKT"] = np.empty((P, Q, G, G), np.float32)
    _RUN["A0"] = np.empty((P, Q, G, G), np.float32)
    _RUN["A1T"] = np.empty((P, Q, G, G), np.float32)
    _RUN["S0"] = np.empty((P, Q, G), np.float32)
    _RUN["S1"] = np.empty((P, Q, G), np.float32)
    _RUN["XIN"] = np.empty((GR, 2 * H), f8)
    _RUN["TAIL"] = np.empty((GR, TAILW), bf16)
    _RUN["OUT"] = np.empty((P, A, O), np.float32)
    _RUN["BMAP"] = (
        np.arange(O)[None, :] // G == np.arange(Q)[:, None]
    ).astype(np.float32)
    return _RUN


def _get_runner():
    st = _get_state()
    if "fn" in st:
        return st["fn"]

    import jax
    import concourse.mybir as mybir
    from concourse.bass2jax import (
        install_neuronx_cc_hook,
        _bass_exec_p,
        partition_id_tensor,
    )
    from jax.sharding import Mesh, PartitionSpec
    from jax.experimental.shard_map import shard_map

    install_neuronx_cc_hook()
    nc = _build_nc()

    partition_name = nc.partition_id_tensor.name if nc.partition_id_tensor else None
    in_names, out_names, out_avals = [], [], []
    for alloc in nc.m.functions[0].allocations:
        if not isinstance(alloc, mybir.MemoryLocationSet):
            continue
        name = alloc.memorylocations[0].name
        if alloc.kind == "ExternalInput":
            if name != partition_name:
                in_names.append(name)
        elif alloc.kind == "ExternalOutput":
            out_names.append(name)
            out_avals.append(
                jax.core.ShapedArray(
                    tuple(alloc.tensor_shape), mybir.dt.np(alloc.dtype)
                )
            )
    # The NEFF/PJRT binding expects one HLO parameter per in_names entry
    # (outputs ride along as donated zero buffers, per run_bass_via_pjrt).
    # partition_id is supplied in-body via PartitionIdOp, last in name order.
    n_params = len(in_names)
    n_outs = len(out_names)
    all_names = tuple(in_names) + tuple(out_names)
    if partition_name is not None:
        all_names = all_names + (partition_name,)
    donate = tuple(range(n_params, n_params + n_outs))

    def _body(*args):
        operands = list(args)
        if partition_name is not None:
            operands.append(partition_id_tensor())
        outs = _bass_exec_p.bind(
            *operands,
            out_avals=tuple(out_avals),
            in_names=all_names,
            out_names=tuple(out_names),
            lowering_input_output_aliases=(),
            sim_require_finite=True,
            sim_require_nnan=True,
            nc=nc,
        )
        return tuple(outs)

    devices = jax.devices()[:NCORES]
    mesh = Mesh(np.asarray(devices), ("core",))
    spec = PartitionSpec("core")
    fn = jax.jit(
        shard_map(
            _body,
            mesh=mesh,
            in_specs=(spec,) * (n_params + n_outs),
            out_specs=(spec,) * n_outs,
            check_rep=False,
        ),
        donate_argnums=donate,
        keep_unused=True,
    )
    st["zeros"] = [
        np.zeros((NCORES * a.shape[0], *a.shape[1:]), a.dtype) for a in out_avals
    ]
    # AOT-lower+compile to skip the pjit python dispatch path on every call
    try:
        import jax as _jax

        arg_structs = []
        for alloc_names, avals in ((in_names, None),):
            pass
        gshapes = []
        for alloc in nc.m.functions[0].allocations:
            if not isinstance(alloc, mybir.MemoryLocationSet):
                continue
            name = alloc.memorylocations[0].name
            if alloc.kind == "ExternalInput" and name != partition_name:
                gshapes.append(
                    _jax.ShapeDtypeStruct(
                        (NCORES * alloc.tensor_shape[0], *alloc.tensor_shape[1:]),
                        mybir.dt.np(alloc.dtype),
                    )
                )
        gshapes += [
            _jax.ShapeDtypeStruct(z.shape, z.dtype) for z in st["zeros"]
        ]
        fn = fn.lower(*gshapes).compile()
    except Exception:
        pass
    st["in_names"] = in_names
    st["fn"] = fn
    return fn


def _prep_inputs(log_prior, ll, quant):
    """Host-side layout prep: fills the cached XIN (fp8) / TAIL (bf16)."""
    st = _get_state()
    BLK, BLKT, A0, A1T = st["BLK"], st["BLKT"], st["A0"], st["A1T"]
    XIN, TAIL = st["XIN"], st["TAIL"]

    prior0 = log_prior[:, 0, :]  # [P, O]
    prior1 = log_prior[:, 1, :]
    llf = ll.reshape(P, O, O)
    i4 = llf.itemsize
    # diagonal 16x16 blocks as zero-copy strided views:
    #   BLK[p,q,r,c]  = ll[p, 16q+r, 16q+c];  BLKT swaps r/c strides.
    bs = (O * O * i4, (G * O + G) * i4, O * i4, i4)
    blkv = as_strided(llf, (P, Q, G, G), bs)
    blkvT = as_strided(llf, (P, Q, G, G), (bs[0], bs[1], bs[3], bs[2]))
    np.minimum(blkv, 0.0, out=BLK)
    np.minimum(blkvT, 0.0, out=BLKT)

    # priors broadcast along the reduced dim (j); both layouts broadcast on
    # their last axis
    np.add(BLK, prior1.reshape(P, Q, 1, G), out=A0)    # [p,q,r,c] + p1[p,16q+c]
    np.add(BLKT, prior0.reshape(P, Q, 1, G), out=A1T)  # [p,q,c,r] + p0[p,16q+r]

    ii = np.arange(G)
    s0, s1 = st["S0"], st["S1"]
    np.sum(A0, axis=3, out=s0)
    s0 -= A0[:, :, ii, ii]   # off-diagonal sums for the alpha=0 linear path
    np.sum(A1T, axis=3, out=s1)
    s1 -= A1T[:, :, ii, ii]
    A0[:, :, ii, ii] = POISON
    A1T[:, :, ii, ii] = POISON

    # cast+write into the global device layout (fp8).  Rows are (p, idx):
    # branch0 idx=r (from A0), branch1 idx=c (from A1T); cols are (q, other).
    e1 = XIN.itemsize * 2 * H  # row stride in bytes (fp8 itemsize = 1)
    v0 = as_strided(XIN, (P, Q, G, G), (G * e1, G, e1, 1))
    v0[...] = A0
    v1 = as_strided(XIN[:, H:], (P, Q, G, G), (G * e1, G, e1, 1))
    v1[...] = A1T

    ab0 = quant[:, 1]  # alpha for branch a=0 (j=2)
    ab1 = quant[:, 0]  # alpha for branch a=1 (j=1)
    # base = (1-alpha)*sum_offdiag + prior_i, at tail cols [branch*Q + q],
    # rows (p, g)
    base0 = (1.0 - ab0)[:, None, None] * s0 + prior0.reshape(P, Q, G)
    base1 = (1.0 - ab1)[:, None, None] * s1 + prior1.reshape(P, Q, G)
    t3 = TAIL.reshape(P, G, TAILW)
    t3[:, :, 0:Q] = base0.transpose(0, 2, 1)
    t3[:, :, Q : 2 * Q] = base1.transpose(0, 2, 1)
    t3[:, :, 2 * Q] = ab0[:, None]
    t3[:, :, 2 * Q + 1] = ab1[:, None]
    return XIN, TAIL


def _assemble(res_g):
    """res_g [1024, 128] fp32 -> out [P, A, O]."""
    st = _get_state()
    out = st["OUT"]
    r4 = res_g.reshape(P, G, 2, Q)
    o4 = out.reshape(P, 2, Q, G)
    o4[:, 0] = r4[:, :, 0, :].transpose(0, 2, 1)
    o4[:, 1] = r4[:, :, 1, :].transpose(0, 2, 1)
    return out


# ---------------------------------------------------------------------------
# Fallback: faithful numpy port of the reference, used only if the inputs do
# not match the hardcoded structure (block-diagonal bmap, dims=[0,1], binary
# quantifiers).  Slow but correct for arbitrary inputs.
# ---------------------------------------------------------------------------

def _pnot_np(x, alpha):
    ex = np.exp(np.minimum(x, np.float32(0.0)))
    lg = np.log(np.clip(np.float32(1.0) - ex, np.float32(1e-12), None))
    return (alpha * lg + (np.float32(1.0) - alpha) * x).astype(np.float32)


def _reference_numpy(log_prior, ll4, quant, dims, bmap):
    ll = np.minimum(ll4.mean(axis=-1, dtype=np.float32), np.float32(0.0))
    diag = np.arange(O)
    out = np.zeros((P, A, O), dtype=np.float32)
    for a in range(2):
        i = dims[a] + 1
        j = dims[1 - a] + 1
        qj = quant[:, j - 1][:, None, None].astype(np.float32)
        if j == 1:
            lp = ll + log_prior[:, 0, :][:, :, None]
        else:
            lp = ll + log_prior[:, 1, :][:, None, :]
        lp = _pnot_np(lp, qj)
        lp[:, diag, diag] = 0.0
        if j == 1:
            lp = np.einsum("qo,pon->pqn", bmap, lp).astype(np.float32)
        else:
            lp = np.einsum("qo,pno->pnq", bmap, lp).astype(np.float32)
        lp = _pnot_np(lp, qj)
        if i == 1:
            lp = lp + log_prior[:, 0, :][:, :, None]
        else:
            lp = lp + log_prior[:, 1, :][:, None, :]
        if i == 2:
            lp = np.transpose(lp, (0, 2, 1))
        out[:, i - 1, :] = (lp * bmap.T[None, :, :]).sum(axis=2)
    return out


def kernel(log_prior, log_likelihood, quantifiers, dim_order, batch_object_map):
    log_prior = np.asarray(log_prior, dtype=np.float32)
    ll = np.asarray(log_likelihood, dtype=np.float32)
    quant = np.asarray(quantifiers, dtype=np.float32)
    dims = [int(v) for v in np.asarray(dim_order)]
    bmap = np.asarray(batch_object_map, dtype=np.float32)

    expected_bmap = _get_state()["BMAP"]
    structured = (
        log_prior.shape == (P, A, O)
        and ll.shape == (P, O, O, 1)
        and quant.shape == (Q, A)
        and bmap.shape == (Q, O)
        and dims == [0, 1]
        and np.array_equal(bmap, expected_bmap)
        and bool(np.all((quant == 0.0) | (quant == 1.0)))
    )
    if not structured:
        return _reference_numpy(log_prior, ll, quant, dims, bmap)

    fn = _get_runner()
    xin, tail = _prep_inputs(log_prior, ll, quant)
    out = fn(xin, tail, *_RUN["zeros"])
    # asarray immediately: the d2h fetch piggybacks on the dispatch RTT
    res_g = np.asarray(out[0])
    return _assemble(res_g)
